# revision 3
# baseline (speedup 1.0000x reference)
"""Canny edge filter on 8 Trainium2 NeuronCores (Bass/Tile).

Batch-parallel: 4 images per core. Per image: 9 row-strips of 128 rows
(stride 120, last at 896) x 3 col-tiles. Convolutions and row-shifts run on
the PE as banded-matrix matmuls (horizontal taps via PSUM-offset
accumulation); NMS bin select uses predicated copies; thresholds are applied
in a second SBUF-resident pass after the per-image max reduce.

Works in the scale-invariant squared domain: no sqrt / atan2 / normalization
(the output only depends on comparisons invariant under those transforms).
"""
import sys

sys.path.insert(0, "/opt/trn_rl_repo")

import numpy as np

import concourse.bass as bass
import concourse.mybir as mybir
import concourse.tile as tile
from concourse.bass_utils import run_bass_kernel_spmd
from concourse.vector_clock import ScopedClock

f32 = np.float32
dt = mybir.dt
Alu = mybir.AluOpType
Act = mybir.ActivationFunctionType

N_CORES = 8
H = W = 1024
P = 128
ROW_STARTS = [0, 120, 240, 360, 480, 600, 720, 840, 896]
# valid output-partition range per strip
VALID = [(0, 124)] + [(4, 124)] * 7 + [(68, 128)]
COL_STARTS = [0, 344, 688]
COL_W = [344, 344, 336]
T22 = float(np.float64(np.tan(np.deg2rad(22.5))))
HI2 = float(np.float32(0.15) * np.float32(0.15))
LO2 = float(np.float32(0.05) * np.float32(0.05))

# ---------------------------------------------------------------- tile patch


def _patched_drain_and_barrier(self, tick_clock, wait_clock):
    nc = self.nc
    drain_inst = nc.sync.drain()
    wait_clock.add_sem_waits(
        drain_inst.ins, ScopedClock({None: tick_clock.global_clock})
    )
    si = drain_inst.ins.sync_info
    waits = list(si.on_wait or []) if si is not None else []
    if len(waits) > 1:
        si.on_wait = waits[:1]
        drain_inst.ins.sync_info = si
        by_id = dict(self.sems.allocated())
        for w in waits[1:]:
            h = by_id.get(w.id)
            if h is None:
                for hh in by_id.values():
                    if getattr(hh, "name", None) == w.ant_name:
                        h = hh
                        break
            assert h is not None, f"cannot resolve semaphore for wait {w}"
            nc.sync.wait_ge(h, w.wait_value)
    nc.all_engine_barrier()
    popped = nc._tile_sem_poison_stack.pop()
    assert popped is self._sem_poison
    nc.clear_and_free_semaphores(list(self.sems.allocated().values()))
    nc.all_engine_barrier()


def _split_excess_waits(nc):
    n_split = 0
    for fn in nc.m.functions:
        for bb in fn.blocks:
            insts = list(bb.instructions)
            out = []
            changed = False
            for ins in insts:
                si = ins.sync_info
                waits = list(si.on_wait) if si is not None and si.on_wait else []
                cap = 2 if isinstance(ins, mybir.InstEventSemaphore) else 1
                if len(waits) > cap:
                    changed = True
                    n_split += 1
                    excess = waits[:-cap]
                    for ci, w in enumerate(excess):
                        nop = mybir.InstNoOp(
                            name=f"{ins.name}-wsplit-{ci}", ins=[], outs=[]
                        )
                        nop.debug = ins.debug
                        nop.engine = ins.engine
                        nop.sync_info = mybir.SyncInfo(on_wait=[w], on_update=[])
                        nc.register_instruction(nop, overwrite=True)
                        out.append(nop)
                    si.on_wait = waits[-cap:]
                    ins.sync_info = si
                out.append(ins)
            if changed:
                bb.instructions = out
    return n_split


tile.TileContext._drain_and_barrier = _patched_drain_and_barrier

# ------------------------------------------------------------- band matrices


def _gauss_k1():
    ax = np.arange(5, dtype=np.float64) - 2.0
    k1 = np.exp(-(ax**2) / 2.0)
    k1 = k1 / k1.sum()
    return k1.astype(f32).astype(np.float64)


def _refl(k, variant):
    """Map raw input-row index to a strip partition (None = drop tap)."""
    if 0 <= k < P:
        return k
    if k < 0 and variant == 0:  # image top: reflect
        return -k
    if k > P - 1 and variant == 2:  # image bottom: reflect
        return 2 * (P - 1) - k
    return None


def _band(taps, variant):
    """taps: {row_offset: coeff}; returns [K=P, M=P] lhsT band matrix."""
    m = np.zeros((P, P), dtype=np.float64)
    for i in range(P):
        for a, c in taps.items():
            k = _refl(i + a, variant)
            if k is not None:
                m[k, i] += c
    return m.astype(f32)


def make_consts():
    k1 = _gauss_k1()
    consts = {}
    for v in range(3):
        vband = _band({a - 2: k1[a] for a in range(5)}, v)
        for j, scale in enumerate([k1[0], k1[1], k1[2]]):
            consts[f"bv{j}_{v}"] = (vband * f32(scale)).astype(f32)
        s = _band({-1: 1.0, 0: 2.0, 1: 1.0}, v)
        consts[f"sb_{v}"] = s
        consts[f"sn_{v}"] = (-s).astype(f32)
        d = _band({-1: 1.0, 1: -1.0}, v)
        consts[f"db_{v}"] = d
        consts[f"d2_{v}"] = (2.0 * d).astype(f32)
    up = np.zeros((P, P), dtype=f32)  # gU[i] = g2[i-1]
    dn = np.zeros((P, P), dtype=f32)  # gD[i] = g2[i+1]
    for i in range(P):
        if i >= 1:
            up[i - 1, i] = 1.0
        if i <= P - 2:
            dn[i + 1, i] = 1.0
    consts["up"] = up
    consts["dn"] = dn
    consts["ones1"] = np.ones((1, P), dtype=f32)
    for v in range(3):
        vm = np.zeros((P, 1), dtype=f32)
        p0, p1 = [(0, 124), (4, 124), (68, 128)][v]
        vm[p0:p1] = 1.0
        consts[f"vm_{v}"] = vm
    return consts


# ------------------------------------------------------------ kernel builder


def build_program(n_img):
    consts = make_consts()
    nc = bass.Bass("TRN2", target_bir_lowering=False, debug=False)
    x_d = nc.dram_tensor("x", [n_img, H, W], dt.float32, kind="ExternalInput").ap()
    o_d = nc.dram_tensor("out", [n_img, H, W], dt.float32, kind="ExternalOutput").ap()
    c_d = {
        k: nc.dram_tensor(k, list(v.shape), dt.float32, kind="ExternalInput").ap()
        for k, v in consts.items()
    }

    with tile.TileContext(nc) as tc:
        with (
            tc.tile_pool(name="cpool", bufs=1) as cpool,
            tc.tile_pool(name="xin", bufs=3) as xin,
            tc.tile_pool(name="work", bufs=2) as work,
            tc.tile_pool(name="nmsbuf", bufs=2) as nmsbuf,
            tc.tile_pool(name="small", bufs=2) as small,
            tc.tile_pool(name="p_xb", bufs=2, space="PSUM") as p_xb,
            tc.tile_pool(name="p_ixy", bufs=1, space="PSUM") as p_ixy,
            tc.tile_pool(name="p_ud", bufs=1, space="PSUM") as p_ud,
            tc.tile_pool(name="p_bc", bufs=1, space="PSUM") as p_bc,
        ):
            cts = {}
            for k, v in consts.items():
                t = cpool.tile(list(v.shape), dt.float32, tag=f"c_{k}")
                nc.sync.dma_start(t[:], c_d[k][:])
                cts[k] = t

            for img in range(n_img):
                # nms^2 strip buffers for this image
                nmsb = []
                for s in range(9):
                    nb = nmsbuf.tile([P, W], dt.float32, tag=f"nmsb{s}")
                    nmsb.append(nb)
                pm = small.tile([P, 16], dt.float32, tag="pm")
                nc.vector.memset(pm[:], 0.0)

                for s in range(9):
                    r0 = ROW_STARTS[s]
                    v = 0 if s == 0 else (2 if s == 8 else 1)
                    p0, p1 = VALID[s]
                    for ci in range(3):
                        c0 = COL_STARTS[ci]
                        Wt = COL_W[ci]
                        xw = Wt + 8
                        # ---- load x tile (cols c0-4 .. c0+Wt+4), reflect at edges
                        xt = xin.tile([P, xw], dt.float32, tag="xt")
                        if ci == 0:
                            nc.sync.dma_start(
                                xt[:, 4:xw], x_d[img, r0 : r0 + P, 0 : Wt + 4]
                            )
                            for kk in range(1, 5):  # pos 4-kk <- pos 4+kk
                                nc.scalar.copy(
                                    xt[:, 4 - kk : 5 - kk], xt[:, 4 + kk : 5 + kk]
                                )
                        elif ci == 2:
                            nc.sync.dma_start(
                                xt[:, 0 : Wt + 4],
                                x_d[img, r0 : r0 + P, c0 - 4 : c0 + Wt],
                            )
                            base = Wt + 3  # pos of col 1023
                            for kk in range(1, 5):  # pos base+kk <- base-kk
                                nc.scalar.copy(
                                    xt[:, base + kk : base + kk + 1],
                                    xt[:, base - kk : base - kk + 1],
                                )
                        else:
                            nc.sync.dma_start(
                                xt[:], x_d[img, r0 : r0 + P, c0 - 4 : c0 + xw - 4]
                            )

                        # ---- blur: xb[q] (cols c0-2+q, q in [0, Wt+4))
                        wxb = Wt + 4
                        xb = p_xb.tile([P, wxb], dt.float32, tag="xb")
                        for bi, (mat, off) in enumerate(
                            [("bv0", 0), ("bv1", 1), ("bv2", 2), ("bv1", 3), ("bv0", 4)]
                        ):
                            nc.tensor.matmul(
                                xb[:],
                                cts[f"{mat}_{v}"][:],
                                xt[:, off : off + wxb],
                                start=(bi == 0),
                                stop=(bi == 4),
                            )
                        xbs = work.tile([P, wxb], dt.float32, tag="xbs")
                        nc.scalar.copy(xbs[:], xb[:])

                        # ---- sobel: ix/iy on cols c0-1 .. c0+Wt+1
                        wg = Wt + 2
                        ixp = p_ixy.tile([P, wg], dt.float32, tag="ixp")
                        nc.tensor.matmul(
                            ixp[:], cts[f"sb_{v}"][:], xbs[:, 2 : 2 + wg],
                            start=True, stop=False,
                        )
                        nc.tensor.matmul(
                            ixp[:], cts[f"sn_{v}"][:], xbs[:, 0:wg],
                            start=False, stop=True,
                        )
                        iyp = p_ixy.tile([P, wg], dt.float32, tag="iyp")
                        nc.tensor.matmul(
                            iyp[:], cts[f"db_{v}"][:], xbs[:, 0:wg],
                            start=True, stop=False,
                        )
                        nc.tensor.matmul(
                            iyp[:], cts[f"d2_{v}"][:], xbs[:, 1 : 1 + wg],
                            start=False, stop=False,
                        )
                        nc.tensor.matmul(
                            iyp[:], cts[f"db_{v}"][:], xbs[:, 2 : 2 + wg],
                            start=False, stop=True,
                        )

                        # ---- squares / gradient magnitude^2
                        ixs = work.tile([P, wg], dt.float32, tag="ixs")
                        nc.scalar.copy(ixs[:], ixp[:])
                        gx = work.tile([P, wg], dt.float32, tag="gx")
                        nc.scalar.activation(gx[:], ixs[:], Act.Square)
                        gxt = work.tile([P, wg], dt.float32, tag="gxt")
                        nc.scalar.activation(gxt[:], ixs[:], Act.Square, scale=T22)
                        gy = work.tile([P, wg], dt.float32, tag="gy")
                        nc.scalar.activation(gy[:], iyp[:], Act.Square)
                        gyt = work.tile([P, wg], dt.float32, tag="gyt")
                        nc.scalar.activation(gyt[:], iyp[:], Act.Square, scale=T22)
                        g2 = work.tile([P, wg], dt.float32, tag="g2")
                        nc.vector.tensor_tensor(g2[:], gx[:], gy[:], Alu.add)
                        if ci == 0:
                            nc.vector.memset(g2[:, 0:1], 0.0)  # col -1 zero pad
                        if ci == 2:
                            nc.vector.memset(g2[:, wg - 1 : wg], 0.0)  # col 1024

                        # ---- NMS row-shifted copies
                        gup = p_ud.tile([P, wg], dt.float32, tag="gup")
                        nc.tensor.matmul(gup[:], cts["up"][:], g2[:], start=True, stop=True)
                        gdp = p_ud.tile([P, wg], dt.float32, tag="gdp")
                        nc.tensor.matmul(gdp[:], cts["dn"][:], g2[:], start=True, stop=True)
                        gus = work.tile([P, wg], dt.float32, tag="gus")
                        nc.scalar.copy(gus[:], gup[:])

                        # ---- neighbor maxima (width Wt, cols c0..c0+Wt)
                        m0 = work.tile([P, Wt], dt.float32, tag="m0")
                        nc.vector.tensor_tensor(m0[:], g2[:, 0:Wt], g2[:, 2 : 2 + Wt], Alu.max)
                        m90 = work.tile([P, Wt], dt.float32, tag="m90")
                        nc.vector.tensor_tensor(m90[:], gus[:, 1 : 1 + Wt], gdp[:, 1 : 1 + Wt], Alu.max)
                        m45 = work.tile([P, Wt], dt.float32, tag="m45")
                        nc.vector.tensor_tensor(m45[:], gus[:, 2 : 2 + Wt], gdp[:, 0:Wt], Alu.max)
                        m135 = work.tile([P, Wt], dt.float32, tag="m135")
                        nc.vector.tensor_tensor(m135[:], gus[:, 0:Wt], gdp[:, 2 : 2 + Wt], Alu.max)

                        # ---- direction masks
                        c0m = work.tile([P, Wt], dt.float32, tag="c0m")
                        nc.vector.tensor_tensor(
                            c0m[:], gy[:, 1 : 1 + Wt], gxt[:, 1 : 1 + Wt], Alu.is_lt
                        )
                        c90m = work.tile([P, Wt], dt.float32, tag="c90m")
                        nc.vector.tensor_tensor(
                            c90m[:], gx[:, 1 : 1 + Wt], gyt[:, 1 : 1 + Wt], Alu.is_le
                        )
                        sm = work.tile([P, Wt], dt.float32, tag="sm")
                        nc.vector.tensor_tensor(
                            sm[:], ixs[:, 1 : 1 + Wt], iyp[:, 1 : 1 + Wt], Alu.mult
                        )
                        s45 = work.tile([P, Wt], dt.float32, tag="s45")
                        nc.vector.tensor_scalar(
                            s45[:], sm[:], 0.0, 1.0, Alu.is_ge, Alu.mult
                        )

                        # ---- bin-select M then keep/nms2
                        M = work.tile([P, Wt], dt.float32, tag="M")
                        nc.vector.tensor_copy(M[:], m135[:])
                        nc.vector.copy_predicated(M[:], s45[:].bitcast(dt.int32), m45[:])
                        nc.vector.copy_predicated(M[:], c90m[:].bitcast(dt.int32), m90[:])
                        nc.vector.copy_predicated(M[:], c0m[:].bitcast(dt.int32), m0[:])
                        keep = work.tile([P, Wt], dt.float32, tag="keep")
                        nc.vector.tensor_tensor(keep[:], g2[:, 1 : 1 + Wt], M[:], Alu.is_ge)
                        nc.vector.tensor_tensor(
                            nmsb[s][:, c0 : c0 + Wt], g2[:, 1 : 1 + Wt], keep[:], Alu.mult
                        )

                    # zero out invalid rows, then per-strip max
                    nc.vector.tensor_scalar(
                        nmsb[s][:], nmsb[s][:], cts[f"vm_{v}"][:], 1.0,
                        Alu.mult, Alu.mult,
                    )
                    nc.vector.tensor_reduce(
                        pm[:, s : s + 1], nmsb[s][:], mybir.AxisListType.X, Alu.max
                    )

                # ---- global max -> thresholds
                pm1 = small.tile([P, 1], dt.float32, tag="pm1")
                nc.vector.tensor_reduce(pm1[:], pm[:], mybir.AxisListType.X, Alu.max)
                pmc = small.tile([1, 1], dt.float32, tag="pmc")
                nc.gpsimd.tensor_reduce(pmc[0:1, :], pm1[:], mybir.AxisListType.C, Alu.max)
                pbc = p_bc.tile([P, 1], dt.float32, tag="pbc")
                nc.tensor.matmul(pbc[:], cts["ones1"][:], pmc[0:1, :], start=True, stop=True)
                hi2 = small.tile([P, 1], dt.float32, tag="hi2")
                nc.scalar.activation(hi2[:], pbc[:], Act.Copy, scale=HI2)
                lo2 = small.tile([P, 1], dt.float32, tag="lo2")
                nc.scalar.activation(lo2[:], pbc[:], Act.Copy, scale=LO2)

                # ---- pass 2: double threshold + store
                for s in range(9):
                    p0, p1 = VALID[s]
                    r0 = ROW_STARTS[s]
                    hi = work.tile([P, W], dt.float32, tag="hi")
                    nc.vector.tensor_scalar(
                        hi[:], nmsb[s][:], hi2[:], 230.0, Alu.is_ge, Alu.mult
                    )
                    lo = work.tile([P, W], dt.float32, tag="lo")
                    nc.vector.tensor_scalar(
                        lo[:], nmsb[s][:], lo2[:], 25.0, Alu.is_ge, Alu.mult
                    )
                    ot = work.tile([P, W], dt.float32, tag="ot")
                    nc.vector.tensor_tensor(ot[:], hi[:], lo[:], Alu.add)
                    nc.sync.dma_start(
                        o_d[img, r0 + p0 : r0 + p1, :], ot[p0:p1, :]
                    )

    _split_excess_waits(nc)
    return nc, consts


_CACHE = {}


def _get_program(n_img):
    if n_img not in _CACHE:
        _CACHE[n_img] = build_program(n_img)
    return _CACHE[n_img]


def kernel(x: np.ndarray) -> np.ndarray:
    B = x.shape[0]
    n_img = B // N_CORES
    nc, consts = _get_program(n_img)
    xs = np.ascontiguousarray(x.reshape(N_CORES, n_img, H, W).astype(np.float32))
    in_maps = []
    for c in range(N_CORES):
        m = {"x": xs[c]}
        m.update(consts)
        in_maps.append(m)
    res = run_bass_kernel_spmd(nc, in_maps, core_ids=list(range(N_CORES)))
    out = np.stack([res.results[c]["out"] for c in range(N_CORES)], axis=0)
    return out.reshape(B, 1, H, W)


if __name__ == "__main__":
    rng = np.random.default_rng(3)
    x = rng.random((8, 1, H, W), dtype=np.float32)  # 1 img/core quick check
    out = kernel(x)
    sys.path.insert(0, "/root/problem")
    from canny_np import ref_canny_np

    num = den = 0.0
    mism = 0
    for b in range(8):
        e = ref_canny_np(x[b, 0])
        mism += int((out[b, 0] != e).sum())
        num += ((out[b, 0] - e) ** 2).sum()
        den += (e**2).sum()
    print(f"mismatched pixels: {mism} / {8 * H * W}")
    print("rel err:", np.sqrt(num / den))


# revision 4
# speedup vs baseline: 1.7027x; 1.7027x over previous
"""Canny edge filter on 8 Trainium2 NeuronCores (Bass/Tile).

Batch-parallel: 4 images per core. Per image: 9 row-strips of 128 rows
(stride 120, last at 896) x 3 col-tiles. Convolutions and row-shifts run on
the PE as banded-matrix matmuls (horizontal taps via PSUM-offset
accumulation); NMS bin select uses predicated copies; thresholds are applied
in a second SBUF-resident pass after the per-image max reduce.

Works in the scale-invariant squared domain: no sqrt / atan2 / normalization
(the output only depends on comparisons invariant under those transforms).
"""
import sys

sys.path.insert(0, "/opt/trn_rl_repo")

import numpy as np

import concourse.bass as bass
import concourse.mybir as mybir
import concourse.tile as tile
from concourse.bass_utils import run_bass_kernel_spmd
from concourse.vector_clock import ScopedClock

f32 = np.float32
dt = mybir.dt
Alu = mybir.AluOpType
Act = mybir.ActivationFunctionType

N_CORES = 8
H = W = 1024
P = 128
ROW_STARTS = [0, 120, 240, 360, 480, 600, 720, 840, 896]
# valid output-partition range per strip
VALID = [(0, 124)] + [(4, 124)] * 7 + [(68, 128)]
COL_STARTS = [0, 344, 688]
COL_W = [344, 344, 336]
T22 = float(np.float64(np.tan(np.deg2rad(22.5))))
HI2 = float(np.float32(0.15) * np.float32(0.15))
LO2 = float(np.float32(0.05) * np.float32(0.05))

# ---------------------------------------------------------------- tile patch


def _patched_drain_and_barrier(self, tick_clock, wait_clock):
    nc = self.nc
    drain_inst = nc.sync.drain()
    wait_clock.add_sem_waits(
        drain_inst.ins, ScopedClock({None: tick_clock.global_clock})
    )
    si = drain_inst.ins.sync_info
    waits = list(si.on_wait or []) if si is not None else []
    if len(waits) > 1:
        si.on_wait = waits[:1]
        drain_inst.ins.sync_info = si
        by_id = dict(self.sems.allocated())
        for w in waits[1:]:
            h = by_id.get(w.id)
            if h is None:
                for hh in by_id.values():
                    if getattr(hh, "name", None) == w.ant_name:
                        h = hh
                        break
            assert h is not None, f"cannot resolve semaphore for wait {w}"
            nc.sync.wait_ge(h, w.wait_value)
    nc.all_engine_barrier()
    popped = nc._tile_sem_poison_stack.pop()
    assert popped is self._sem_poison
    nc.clear_and_free_semaphores(list(self.sems.allocated().values()))
    nc.all_engine_barrier()


def _split_excess_waits(nc):
    n_split = 0
    for fn in nc.m.functions:
        for bb in fn.blocks:
            insts = list(bb.instructions)
            out = []
            changed = False
            for ins in insts:
                si = ins.sync_info
                waits = list(si.on_wait) if si is not None and si.on_wait else []
                cap = 2 if isinstance(ins, mybir.InstEventSemaphore) else 1
                if len(waits) > cap:
                    changed = True
                    n_split += 1
                    excess = waits[:-cap]
                    for ci, w in enumerate(excess):
                        nop = mybir.InstNoOp(
                            name=f"{ins.name}-wsplit-{ci}", ins=[], outs=[]
                        )
                        nop.debug = ins.debug
                        nop.engine = ins.engine
                        nop.sync_info = mybir.SyncInfo(on_wait=[w], on_update=[])
                        nc.register_instruction(nop, overwrite=True)
                        out.append(nop)
                    si.on_wait = waits[-cap:]
                    ins.sync_info = si
                out.append(ins)
            if changed:
                bb.instructions = out
    return n_split


tile.TileContext._drain_and_barrier = _patched_drain_and_barrier

# ------------------------------------------------------------- band matrices


def _gauss_k1():
    ax = np.arange(5, dtype=np.float64) - 2.0
    k1 = np.exp(-(ax**2) / 2.0)
    k1 = k1 / k1.sum()
    return k1.astype(f32).astype(np.float64)


def _refl(k, variant):
    """Map raw input-row index to a strip partition (None = drop tap)."""
    if 0 <= k < P:
        return k
    if k < 0 and variant == 0:  # image top: reflect
        return -k
    if k > P - 1 and variant == 2:  # image bottom: reflect
        return 2 * (P - 1) - k
    return None


def _band(taps, variant):
    """taps: {row_offset: coeff}; returns [K=P, M=P] lhsT band matrix."""
    m = np.zeros((P, P), dtype=np.float64)
    for i in range(P):
        for a, c in taps.items():
            k = _refl(i + a, variant)
            if k is not None:
                m[k, i] += c
    return m.astype(f32)


def make_consts():
    k1 = _gauss_k1()
    consts = {}
    for v in range(3):
        vband = _band({a - 2: k1[a] for a in range(5)}, v)
        for j, scale in enumerate([k1[0], k1[1], k1[2]]):
            consts[f"bv{j}_{v}"] = (vband * f32(scale)).astype(f32)
        s = _band({-1: 1.0, 0: 2.0, 1: 1.0}, v)
        consts[f"sb_{v}"] = s
        consts[f"sn_{v}"] = (-s).astype(f32)
        d = _band({-1: 1.0, 1: -1.0}, v)
        consts[f"db_{v}"] = d
        consts[f"d2_{v}"] = (2.0 * d).astype(f32)
    up = np.zeros((P, P), dtype=f32)  # gU[i] = g2[i-1]
    dn = np.zeros((P, P), dtype=f32)  # gD[i] = g2[i+1]
    for i in range(P):
        if i >= 1:
            up[i - 1, i] = 1.0
        if i <= P - 2:
            dn[i + 1, i] = 1.0
    consts["up"] = up
    consts["dn"] = dn
    consts["ones1"] = np.ones((1, P), dtype=f32)
    consts["id128"] = np.eye(P, dtype=f32)
    for v in range(3):
        vm = np.zeros((P, 1), dtype=f32)
        p0, p1 = [(0, 124), (4, 124), (68, 128)][v]
        vm[p0:p1] = 1.0
        consts[f"vm_{v}"] = vm
    return consts


# ------------------------------------------------------------ kernel builder


def build_program(n_img):
    consts = make_consts()
    nc = bass.Bass("TRN2", target_bir_lowering=False, debug=False)
    x_d = nc.dram_tensor("x", [n_img, H, W], dt.float32, kind="ExternalInput").ap()
    o_d = nc.dram_tensor("out", [n_img, H, W], dt.float32, kind="ExternalOutput").ap()
    c_d = {
        k: nc.dram_tensor(k, list(v.shape), dt.float32, kind="ExternalInput").ap()
        for k, v in consts.items()
    }

    with tile.TileContext(nc) as tc:
        with (
            tc.tile_pool(name="cpool", bufs=1) as cpool,
            tc.tile_pool(name="xin", bufs=3) as xin,
            tc.tile_pool(name="work", bufs=2) as work,
            tc.tile_pool(name="nmsbuf", bufs=2) as nmsbuf,
            tc.tile_pool(name="small", bufs=2) as small,
            tc.tile_pool(name="p_xb", bufs=2, space="PSUM") as p_xb,
            tc.tile_pool(name="p_ixy", bufs=1, space="PSUM") as p_ixy,
            tc.tile_pool(name="p_ud", bufs=1, space="PSUM") as p_ud,
            tc.tile_pool(name="p_bc", bufs=1, space="PSUM") as p_bc,
        ):
            cts = {}
            for k, v in consts.items():
                t = cpool.tile(list(v.shape), dt.float32, tag=f"c_{k}")
                nc.sync.dma_start(t[:], c_d[k][:])
                cts[k] = t

            for img in range(n_img):
                # nms^2 strip buffers for this image
                nmsb = []
                for s in range(9):
                    nb = nmsbuf.tile([P, W], dt.float32, tag=f"nmsb{s}")
                    nmsb.append(nb)
                pm = small.tile([P, 16], dt.float32, tag="pm")
                nc.vector.memset(pm[:], 0.0)

                for s in range(9):
                    r0 = ROW_STARTS[s]
                    v = 0 if s == 0 else (2 if s == 8 else 1)
                    p0, p1 = VALID[s]
                    for ci in range(3):
                        c0 = COL_STARTS[ci]
                        Wt = COL_W[ci]
                        xw = Wt + 8
                        # ---- load x tile (cols c0-4 .. c0+Wt+4), reflect at edges
                        xt = xin.tile([P, xw], dt.float32, tag="xt")
                        if ci == 0:
                            nc.sync.dma_start(
                                xt[:, 4:xw], x_d[img, r0 : r0 + P, 0 : Wt + 4]
                            )
                            for kk in range(1, 5):  # pos 4-kk <- pos 4+kk
                                nc.scalar.copy(
                                    xt[:, 4 - kk : 5 - kk], xt[:, 4 + kk : 5 + kk]
                                )
                        elif ci == 2:
                            nc.sync.dma_start(
                                xt[:, 0 : Wt + 4],
                                x_d[img, r0 : r0 + P, c0 - 4 : c0 + Wt],
                            )
                            base = Wt + 3  # pos of col 1023
                            for kk in range(1, 5):  # pos base+kk <- base-kk
                                nc.scalar.copy(
                                    xt[:, base + kk : base + kk + 1],
                                    xt[:, base - kk : base - kk + 1],
                                )
                        else:
                            nc.sync.dma_start(
                                xt[:], x_d[img, r0 : r0 + P, c0 - 4 : c0 + xw - 4]
                            )

                        # ---- blur: xb[q] (cols c0-2+q, q in [0, Wt+4))
                        wxb = Wt + 4
                        xb = p_xb.tile([P, wxb], dt.float32, tag="xb")
                        for bi, (mat, off) in enumerate(
                            [("bv0", 0), ("bv1", 1), ("bv2", 2), ("bv1", 3), ("bv0", 4)]
                        ):
                            nc.tensor.matmul(
                                xb[:],
                                cts[f"{mat}_{v}"][:],
                                xt[:, off : off + wxb],
                                start=(bi == 0),
                                stop=(bi == 4),
                            )
                        xbs = work.tile([P, wxb], dt.float32, tag="xbs")
                        nc.scalar.copy(xbs[:], xb[:])

                        # ---- sobel: ix/iy on cols c0-1 .. c0+Wt+1
                        wg = Wt + 2
                        ixp = p_ixy.tile([P, wg], dt.float32, tag="ixp")
                        nc.tensor.matmul(
                            ixp[:], cts[f"sb_{v}"][:], xbs[:, 2 : 2 + wg],
                            start=True, stop=False,
                        )
                        nc.tensor.matmul(
                            ixp[:], cts[f"sn_{v}"][:], xbs[:, 0:wg],
                            start=False, stop=True,
                        )
                        iyp = p_ixy.tile([P, wg], dt.float32, tag="iyp")
                        nc.tensor.matmul(
                            iyp[:], cts[f"db_{v}"][:], xbs[:, 0:wg],
                            start=True, stop=False,
                        )
                        nc.tensor.matmul(
                            iyp[:], cts[f"d2_{v}"][:], xbs[:, 1 : 1 + wg],
                            start=False, stop=False,
                        )
                        nc.tensor.matmul(
                            iyp[:], cts[f"db_{v}"][:], xbs[:, 2 : 2 + wg],
                            start=False, stop=True,
                        )

                        # ---- squares / gradient magnitude^2
                        ixs = work.tile([P, wg], dt.float32, tag="ixs")
                        nc.scalar.copy(ixs[:], ixp[:])
                        gx = work.tile([P, wg], dt.float32, tag="gx")
                        nc.scalar.activation(gx[:], ixs[:], Act.Square)
                        gy = work.tile([P, wg], dt.float32, tag="gy")
                        nc.scalar.activation(gy[:], iyp[:], Act.Square)
                        g2 = work.tile([P, wg], dt.float32, tag="g2")
                        nc.vector.tensor_tensor(g2[:], gx[:], gy[:], Alu.add)
                        if ci == 0:
                            nc.vector.memset(g2[:, 0:1], 0.0)  # col -1 zero pad
                        if ci == 2:
                            nc.vector.memset(g2[:, wg - 1 : wg], 0.0)  # col 1024

                        # ---- NMS row-shifted copies
                        gup = p_ud.tile([P, wg], dt.float32, tag="gup")
                        nc.tensor.matmul(gup[:], cts["up"][:], g2[:], start=True, stop=True)
                        gdp = p_ud.tile([P, wg], dt.float32, tag="gdp")
                        nc.tensor.matmul(gdp[:], cts["dn"][:], g2[:], start=True, stop=True)
                        gus = work.tile([P, wg], dt.float32, tag="gus")
                        nc.scalar.copy(gus[:], gup[:])

                        # ---- neighbor maxima (width Wt, cols c0..c0+Wt)
                        m0 = work.tile([P, Wt], dt.float32, tag="m0")
                        nc.vector.tensor_tensor(m0[:], g2[:, 0:Wt], g2[:, 2 : 2 + Wt], Alu.max)
                        m90 = work.tile([P, Wt], dt.float32, tag="m90")
                        nc.vector.tensor_tensor(m90[:], gus[:, 1 : 1 + Wt], gdp[:, 1 : 1 + Wt], Alu.max)
                        m45 = work.tile([P, Wt], dt.float32, tag="m45")
                        nc.vector.tensor_tensor(m45[:], gus[:, 2 : 2 + Wt], gdp[:, 0:Wt], Alu.max)
                        m135 = work.tile([P, Wt], dt.float32, tag="m135")
                        nc.vector.tensor_tensor(m135[:], gus[:, 0:Wt], gdp[:, 2 : 2 + Wt], Alu.max)

                        # ---- direction masks
                        c0m = work.tile([P, Wt], dt.float32, tag="c0m")
                        nc.vector.scalar_tensor_tensor(
                            c0m[:], gx[:, 1 : 1 + Wt], T22 * T22, gy[:, 1 : 1 + Wt],
                            Alu.mult, Alu.is_gt,
                        )
                        c90m = work.tile([P, Wt], dt.float32, tag="c90m")
                        nc.vector.scalar_tensor_tensor(
                            c90m[:], gy[:, 1 : 1 + Wt], T22 * T22, gx[:, 1 : 1 + Wt],
                            Alu.mult, Alu.is_ge,
                        )
                        sm = work.tile([P, Wt], dt.float32, tag="sm")
                        nc.vector.tensor_tensor(
                            sm[:], ixs[:, 1 : 1 + Wt], iyp[:, 1 : 1 + Wt], Alu.mult
                        )
                        s45 = work.tile([P, Wt], dt.float32, tag="s45")
                        nc.gpsimd.tensor_scalar(
                            s45[:], sm[:], 0.0, 1.0, Alu.is_ge, Alu.mult
                        )

                        # ---- bin-select M then keep/nms2
                        M = work.tile([P, Wt], dt.float32, tag="M")
                        nc.gpsimd.tensor_copy(M[:], m135[:])
                        nc.vector.copy_predicated(M[:], s45[:].bitcast(dt.int32), m45[:])
                        nc.vector.copy_predicated(M[:], c90m[:].bitcast(dt.int32), m90[:])
                        nc.vector.copy_predicated(M[:], c0m[:].bitcast(dt.int32), m0[:])
                        keep = work.tile([P, Wt], dt.float32, tag="keep")
                        nc.vector.tensor_tensor(keep[:], g2[:, 1 : 1 + Wt], M[:], Alu.is_ge)
                        nc.vector.tensor_tensor(
                            nmsb[s][:, c0 : c0 + Wt], g2[:, 1 : 1 + Wt], keep[:], Alu.mult
                        )

                    # zero out invalid rows, then per-strip max
                    nc.gpsimd.tensor_scalar(
                        nmsb[s][:], nmsb[s][:], cts[f"vm_{v}"][:], 1.0,
                        Alu.mult, Alu.mult,
                    )
                    nc.vector.tensor_reduce(
                        pm[:, s : s + 1], nmsb[s][:], mybir.AxisListType.X, Alu.max
                    )

                # ---- global max -> thresholds
                pm1 = small.tile([P, 1], dt.float32, tag="pm1")
                nc.vector.tensor_reduce(pm1[:], pm[:], mybir.AxisListType.X, Alu.max)
                pmt = p_bc.tile([1, P], dt.float32, tag="pmt")
                nc.tensor.transpose(pmt[0:1, :], pm1[:], cts["id128"][:])
                pmc = small.tile([1, 1], dt.float32, tag="pmc")
                nc.vector.tensor_reduce(pmc[0:1, :], pmt[0:1, :], mybir.AxisListType.X, Alu.max)
                pbc = p_bc.tile([P, 1], dt.float32, tag="pbc")
                nc.tensor.matmul(pbc[:], cts["ones1"][:], pmc[0:1, :], start=True, stop=True)
                hi2 = small.tile([P, 1], dt.float32, tag="hi2")
                nc.scalar.activation(hi2[:], pbc[:], Act.Copy, scale=HI2)
                lo2 = small.tile([P, 1], dt.float32, tag="lo2")
                nc.scalar.activation(lo2[:], pbc[:], Act.Copy, scale=LO2)

                # ---- pass 2: double threshold + store
                for s in range(9):
                    p0, p1 = VALID[s]
                    r0 = ROW_STARTS[s]
                    hi = work.tile([P, W], dt.float32, tag="hi")
                    nc.vector.tensor_scalar(
                        hi[:], nmsb[s][:], hi2[:], 230.0, Alu.is_ge, Alu.mult
                    )
                    lo = work.tile([P, W], dt.float32, tag="lo")
                    nc.gpsimd.tensor_scalar(
                        lo[:], nmsb[s][:], lo2[:], 25.0, Alu.is_ge, Alu.mult
                    )
                    ot = work.tile([P, W], dt.float32, tag="ot")
                    nc.vector.tensor_tensor(ot[:], hi[:], lo[:], Alu.add)
                    nc.sync.dma_start(
                        o_d[img, r0 + p0 : r0 + p1, :], ot[p0:p1, :]
                    )

    _split_excess_waits(nc)
    return nc, consts


_CACHE = {}


def _get_program(n_img):
    if n_img not in _CACHE:
        _CACHE[n_img] = build_program(n_img)
    return _CACHE[n_img]


def kernel(x: np.ndarray) -> np.ndarray:
    B = x.shape[0]
    n_img = B // N_CORES
    nc, consts = _get_program(n_img)
    xs = np.ascontiguousarray(x.reshape(N_CORES, n_img, H, W).astype(np.float32))
    in_maps = []
    for c in range(N_CORES):
        m = {"x": xs[c]}
        m.update(consts)
        in_maps.append(m)
    res = run_bass_kernel_spmd(nc, in_maps, core_ids=list(range(N_CORES)))
    out = np.stack([res.results[c]["out"] for c in range(N_CORES)], axis=0)
    return out.reshape(B, 1, H, W)


if __name__ == "__main__":
    rng = np.random.default_rng(3)
    x = rng.random((8, 1, H, W), dtype=np.float32)  # 1 img/core quick check
    out = kernel(x)
    sys.path.insert(0, "/root/problem")
    from canny_np import ref_canny_np

    num = den = 0.0
    mism = 0
    for b in range(8):
        e = ref_canny_np(x[b, 0])
        mism += int((out[b, 0] != e).sum())
        num += ((out[b, 0] - e) ** 2).sum()
        den += (e**2).sum()
    print(f"mismatched pixels: {mism} / {8 * H * W}")
    print("rel err:", np.sqrt(num / den))


# revision 5
# speedup vs baseline: 18.6986x; 10.9818x over previous
"""Canny edge filter on 8 Trainium2 NeuronCores (Bass/Tile).

Batch-parallel: 4 images per core. Per image: 9 row-strips of 128 rows
(stride 120, last at 896) x 3 col-tiles. Convolutions and row-shifts run on
the PE as banded-matrix matmuls (horizontal taps via PSUM-offset
accumulation); NMS bin select uses predicated copies; thresholds are applied
in a second SBUF-resident pass after the per-image max reduce.

Works in the scale-invariant squared domain: no sqrt / atan2 / normalization
(the output only depends on comparisons invariant under those transforms).
"""
import sys

sys.path.insert(0, "/opt/trn_rl_repo")

import numpy as np

import concourse.bass as bass
import concourse.mybir as mybir
import concourse.tile as tile
from concourse.bass_utils import run_bass_kernel_spmd
from concourse.vector_clock import ScopedClock

f32 = np.float32
dt = mybir.dt
Alu = mybir.AluOpType
Act = mybir.ActivationFunctionType

N_CORES = 8
H = W = 1024
P = 128
ROW_STARTS = [0, 120, 240, 360, 480, 600, 720, 840, 896]
# valid output-partition range per strip
VALID = [(0, 124)] + [(4, 124)] * 7 + [(68, 128)]
COL_STARTS = [0, 344, 688]
COL_W = [344, 344, 336]
T22 = float(np.float64(np.tan(np.deg2rad(22.5))))
HI2 = float(np.float32(0.15) * np.float32(0.15))
LO2 = float(np.float32(0.05) * np.float32(0.05))

# ---------------------------------------------------------------- tile patch


def _patched_drain_and_barrier(self, tick_clock, wait_clock):
    nc = self.nc
    drain_inst = nc.sync.drain()
    wait_clock.add_sem_waits(
        drain_inst.ins, ScopedClock({None: tick_clock.global_clock})
    )
    si = drain_inst.ins.sync_info
    waits = list(si.on_wait or []) if si is not None else []
    if len(waits) > 1:
        si.on_wait = waits[:1]
        drain_inst.ins.sync_info = si
        by_id = dict(self.sems.allocated())
        for w in waits[1:]:
            h = by_id.get(w.id)
            if h is None:
                for hh in by_id.values():
                    if getattr(hh, "name", None) == w.ant_name:
                        h = hh
                        break
            assert h is not None, f"cannot resolve semaphore for wait {w}"
            nc.sync.wait_ge(h, w.wait_value)
    nc.all_engine_barrier()
    popped = nc._tile_sem_poison_stack.pop()
    assert popped is self._sem_poison
    nc.clear_and_free_semaphores(list(self.sems.allocated().values()))
    nc.all_engine_barrier()


def _split_excess_waits(nc):
    n_split = 0
    for fn in nc.m.functions:
        for bb in fn.blocks:
            insts = list(bb.instructions)
            out = []
            changed = False
            for ins in insts:
                si = ins.sync_info
                waits = list(si.on_wait) if si is not None and si.on_wait else []
                cap = 2 if isinstance(ins, mybir.InstEventSemaphore) else 1
                if len(waits) > cap:
                    changed = True
                    n_split += 1
                    excess = waits[:-cap]
                    for ci, w in enumerate(excess):
                        nop = mybir.InstNoOp(
                            name=f"{ins.name}-wsplit-{ci}", ins=[], outs=[]
                        )
                        nop.debug = ins.debug
                        nop.engine = ins.engine
                        nop.sync_info = mybir.SyncInfo(on_wait=[w], on_update=[])
                        nc.register_instruction(nop, overwrite=True)
                        out.append(nop)
                    si.on_wait = waits[-cap:]
                    ins.sync_info = si
                out.append(ins)
            if changed:
                bb.instructions = out
    return n_split


tile.TileContext._drain_and_barrier = _patched_drain_and_barrier

# ------------------------------------------------------------- band matrices


def _gauss_k1():
    ax = np.arange(5, dtype=np.float64) - 2.0
    k1 = np.exp(-(ax**2) / 2.0)
    k1 = k1 / k1.sum()
    return k1.astype(f32).astype(np.float64)


def _refl(k, variant):
    """Map raw input-row index to a strip partition (None = drop tap)."""
    if 0 <= k < P:
        return k
    if k < 0 and variant == 0:  # image top: reflect
        return -k
    if k > P - 1 and variant == 2:  # image bottom: reflect
        return 2 * (P - 1) - k
    return None


def _band(taps, variant):
    """taps: {row_offset: coeff}; returns [K=P, M=P] lhsT band matrix."""
    m = np.zeros((P, P), dtype=np.float64)
    for i in range(P):
        for a, c in taps.items():
            k = _refl(i + a, variant)
            if k is not None:
                m[k, i] += c
    return m.astype(f32)


def make_consts():
    k1 = _gauss_k1()
    consts = {}
    for v in range(3):
        vband = _band({a - 2: k1[a] for a in range(5)}, v)
        for j, scale in enumerate([k1[0], k1[1], k1[2]]):
            consts[f"bv{j}_{v}"] = (vband * f32(scale)).astype(f32)
        s = _band({-1: 1.0, 0: 2.0, 1: 1.0}, v)
        consts[f"sb_{v}"] = s
        consts[f"sn_{v}"] = (-s).astype(f32)
        d = _band({-1: 1.0, 1: -1.0}, v)
        consts[f"db_{v}"] = d
        consts[f"d2_{v}"] = (2.0 * d).astype(f32)
    up = np.zeros((P, P), dtype=f32)  # gU[i] = g2[i-1]
    dn = np.zeros((P, P), dtype=f32)  # gD[i] = g2[i+1]
    for i in range(P):
        if i >= 1:
            up[i - 1, i] = 1.0
        if i <= P - 2:
            dn[i + 1, i] = 1.0
    consts["up"] = up
    consts["dn"] = dn
    consts["ones1"] = np.ones((1, P), dtype=f32)
    consts["id128"] = np.eye(P, dtype=f32)
    for v in range(3):
        vm = np.zeros((P, 1), dtype=f32)
        p0, p1 = [(0, 124), (4, 124), (68, 128)][v]
        vm[p0:p1] = 1.0
        consts[f"vm_{v}"] = vm
    return consts


# ------------------------------------------------------------ kernel builder


def build_program(n_img, reps=1):
    consts = make_consts()
    nc = bass.Bass("TRN2", target_bir_lowering=False, debug=False)
    x_d = nc.dram_tensor("x", [n_img, H, W], dt.float32, kind="ExternalInput").ap()
    o_d = nc.dram_tensor("out", [n_img, H, W], dt.float32, kind="ExternalOutput").ap()
    c_d = {
        k: nc.dram_tensor(k, list(v.shape), dt.float32, kind="ExternalInput").ap()
        for k, v in consts.items()
    }

    with tile.TileContext(nc) as tc:
        with (
            tc.tile_pool(name="cpool", bufs=1) as cpool,
            tc.tile_pool(name="xin", bufs=3) as xin,
            tc.tile_pool(name="work", bufs=2) as work,
            tc.tile_pool(name="nmsbuf", bufs=2) as nmsbuf,
            tc.tile_pool(name="small", bufs=2) as small,
            tc.tile_pool(name="p_xb", bufs=2, space="PSUM") as p_xb,
            tc.tile_pool(name="p_ixy", bufs=1, space="PSUM") as p_ixy,
            tc.tile_pool(name="p_ud", bufs=1, space="PSUM") as p_ud,
            tc.tile_pool(name="p_bc", bufs=1, space="PSUM") as p_bc,
        ):
            cts = {}
            for k, v in consts.items():
                t = cpool.tile(list(v.shape), dt.float32, tag=f"c_{k}")
                nc.sync.dma_start(t[:], c_d[k][:])
                cts[k] = t

            for _rep, img in [(r, i) for r in range(reps) for i in range(n_img)]:
                # nms^2 strip buffers for this image
                nmsb = []
                for s in range(9):
                    nb = nmsbuf.tile([P, W], dt.float32, tag=f"nmsb{s}")
                    nmsb.append(nb)
                pm = small.tile([P, 16], dt.float32, tag="pm")
                nc.vector.memset(pm[:], 0.0)

                for s in range(9):
                    r0 = ROW_STARTS[s]
                    v = 0 if s == 0 else (2 if s == 8 else 1)
                    p0, p1 = VALID[s]
                    for ci in range(3):
                        c0 = COL_STARTS[ci]
                        Wt = COL_W[ci]
                        xw = Wt + 8
                        # ---- load x tile (cols c0-4 .. c0+Wt+4), reflect at edges
                        xt = xin.tile([P, xw], dt.float32, tag="xt")
                        if ci == 0:
                            nc.sync.dma_start(
                                xt[:, 4:xw], x_d[img, r0 : r0 + P, 0 : Wt + 4]
                            )
                            for kk in range(1, 5):  # pos 4-kk <- pos 4+kk
                                nc.scalar.copy(
                                    xt[:, 4 - kk : 5 - kk], xt[:, 4 + kk : 5 + kk]
                                )
                        elif ci == 2:
                            nc.sync.dma_start(
                                xt[:, 0 : Wt + 4],
                                x_d[img, r0 : r0 + P, c0 - 4 : c0 + Wt],
                            )
                            base = Wt + 3  # pos of col 1023
                            for kk in range(1, 5):  # pos base+kk <- base-kk
                                nc.scalar.copy(
                                    xt[:, base + kk : base + kk + 1],
                                    xt[:, base - kk : base - kk + 1],
                                )
                        else:
                            nc.sync.dma_start(
                                xt[:], x_d[img, r0 : r0 + P, c0 - 4 : c0 + xw - 4]
                            )

                        # ---- blur: xb[q] (cols c0-2+q, q in [0, Wt+4))
                        wxb = Wt + 4
                        xb = p_xb.tile([P, wxb], dt.float32, tag="xb")
                        for bi, (mat, off) in enumerate(
                            [("bv0", 0), ("bv1", 1), ("bv2", 2), ("bv1", 3), ("bv0", 4)]
                        ):
                            nc.tensor.matmul(
                                xb[:],
                                cts[f"{mat}_{v}"][:],
                                xt[:, off : off + wxb],
                                start=(bi == 0),
                                stop=(bi == 4),
                            )
                        xbs = work.tile([P, wxb], dt.float32, tag="xbs")
                        nc.scalar.copy(xbs[:], xb[:])

                        # ---- sobel: ix/iy on cols c0-1 .. c0+Wt+1
                        wg = Wt + 2
                        ixp = p_ixy.tile([P, wg], dt.float32, tag="ixp")
                        nc.tensor.matmul(
                            ixp[:], cts[f"sb_{v}"][:], xbs[:, 2 : 2 + wg],
                            start=True, stop=False,
                        )
                        nc.tensor.matmul(
                            ixp[:], cts[f"sn_{v}"][:], xbs[:, 0:wg],
                            start=False, stop=True,
                        )
                        iyp = p_ixy.tile([P, wg], dt.float32, tag="iyp")
                        nc.tensor.matmul(
                            iyp[:], cts[f"db_{v}"][:], xbs[:, 0:wg],
                            start=True, stop=False,
                        )
                        nc.tensor.matmul(
                            iyp[:], cts[f"d2_{v}"][:], xbs[:, 1 : 1 + wg],
                            start=False, stop=False,
                        )
                        nc.tensor.matmul(
                            iyp[:], cts[f"db_{v}"][:], xbs[:, 2 : 2 + wg],
                            start=False, stop=True,
                        )

                        # ---- squares / gradient magnitude^2
                        ixs = work.tile([P, wg], dt.float32, tag="ixs")
                        nc.scalar.copy(ixs[:], ixp[:])
                        gx = work.tile([P, wg], dt.float32, tag="gx")
                        nc.scalar.activation(gx[:], ixs[:], Act.Square)
                        gy = work.tile([P, wg], dt.float32, tag="gy")
                        nc.scalar.activation(gy[:], iyp[:], Act.Square)
                        g2 = work.tile([P, wg], dt.float32, tag="g2")
                        nc.vector.tensor_tensor(g2[:], gx[:], gy[:], Alu.add)
                        if ci == 0:
                            nc.vector.memset(g2[:, 0:1], 0.0)  # col -1 zero pad
                        if ci == 2:
                            nc.vector.memset(g2[:, wg - 1 : wg], 0.0)  # col 1024

                        # ---- NMS row-shifted copies
                        gup = p_ud.tile([P, wg], dt.float32, tag="gup")
                        nc.tensor.matmul(gup[:], cts["up"][:], g2[:], start=True, stop=True)
                        gdp = p_ud.tile([P, wg], dt.float32, tag="gdp")
                        nc.tensor.matmul(gdp[:], cts["dn"][:], g2[:], start=True, stop=True)
                        gus = work.tile([P, wg], dt.float32, tag="gus")
                        nc.scalar.copy(gus[:], gup[:])

                        # ---- neighbor maxima (width Wt, cols c0..c0+Wt)
                        m0 = work.tile([P, Wt], dt.float32, tag="m0")
                        nc.vector.tensor_tensor(m0[:], g2[:, 0:Wt], g2[:, 2 : 2 + Wt], Alu.max)
                        m90 = work.tile([P, Wt], dt.float32, tag="m90")
                        nc.vector.tensor_tensor(m90[:], gus[:, 1 : 1 + Wt], gdp[:, 1 : 1 + Wt], Alu.max)
                        m45 = work.tile([P, Wt], dt.float32, tag="m45")
                        nc.vector.tensor_tensor(m45[:], gus[:, 2 : 2 + Wt], gdp[:, 0:Wt], Alu.max)
                        m135 = work.tile([P, Wt], dt.float32, tag="m135")
                        nc.vector.tensor_tensor(m135[:], gus[:, 0:Wt], gdp[:, 2 : 2 + Wt], Alu.max)

                        # ---- direction masks
                        c0m = work.tile([P, Wt], dt.float32, tag="c0m")
                        nc.vector.scalar_tensor_tensor(
                            c0m[:], gx[:, 1 : 1 + Wt], T22 * T22, gy[:, 1 : 1 + Wt],
                            Alu.mult, Alu.is_gt,
                        )
                        c90m = work.tile([P, Wt], dt.float32, tag="c90m")
                        nc.vector.scalar_tensor_tensor(
                            c90m[:], gy[:, 1 : 1 + Wt], T22 * T22, gx[:, 1 : 1 + Wt],
                            Alu.mult, Alu.is_ge,
                        )
                        sm = work.tile([P, Wt], dt.float32, tag="sm")
                        nc.vector.tensor_tensor(
                            sm[:], ixs[:, 1 : 1 + Wt], iyp[:, 1 : 1 + Wt], Alu.mult
                        )
                        s45 = work.tile([P, Wt], dt.float32, tag="s45")
                        nc.gpsimd.tensor_scalar(
                            s45[:], sm[:], 0.0, 1.0, Alu.is_ge, Alu.mult
                        )

                        # ---- bin-select M then keep/nms2
                        M = work.tile([P, Wt], dt.float32, tag="M")
                        nc.gpsimd.tensor_copy(M[:], m135[:])
                        nc.vector.copy_predicated(M[:], s45[:].bitcast(dt.int32), m45[:])
                        nc.vector.copy_predicated(M[:], c90m[:].bitcast(dt.int32), m90[:])
                        nc.vector.copy_predicated(M[:], c0m[:].bitcast(dt.int32), m0[:])
                        keep = work.tile([P, Wt], dt.float32, tag="keep")
                        nc.vector.tensor_tensor(keep[:], g2[:, 1 : 1 + Wt], M[:], Alu.is_ge)
                        nc.vector.tensor_tensor(
                            nmsb[s][:, c0 : c0 + Wt], g2[:, 1 : 1 + Wt], keep[:], Alu.mult
                        )

                    # zero out invalid rows, then per-strip max
                    nc.gpsimd.tensor_scalar(
                        nmsb[s][:], nmsb[s][:], cts[f"vm_{v}"][:], 1.0,
                        Alu.mult, Alu.mult,
                    )
                    nc.vector.tensor_reduce(
                        pm[:, s : s + 1], nmsb[s][:], mybir.AxisListType.X, Alu.max
                    )

                # ---- global max -> thresholds
                pm1 = small.tile([P, 1], dt.float32, tag="pm1")
                nc.vector.tensor_reduce(pm1[:], pm[:], mybir.AxisListType.X, Alu.max)
                pmt = p_bc.tile([1, P], dt.float32, tag="pmt")
                nc.tensor.transpose(pmt[0:1, :], pm1[:], cts["id128"][:])
                pmc = small.tile([1, 1], dt.float32, tag="pmc")
                nc.vector.tensor_reduce(pmc[0:1, :], pmt[0:1, :], mybir.AxisListType.X, Alu.max)
                pbc = p_bc.tile([P, 1], dt.float32, tag="pbc")
                nc.tensor.matmul(pbc[:], cts["ones1"][:], pmc[0:1, :], start=True, stop=True)
                hi2 = small.tile([P, 1], dt.float32, tag="hi2")
                nc.scalar.activation(hi2[:], pbc[:], Act.Copy, scale=HI2)
                lo2 = small.tile([P, 1], dt.float32, tag="lo2")
                nc.scalar.activation(lo2[:], pbc[:], Act.Copy, scale=LO2)

                # ---- pass 2: double threshold + store
                for s in range(9):
                    p0, p1 = VALID[s]
                    r0 = ROW_STARTS[s]
                    hi = work.tile([P, W], dt.float32, tag="hi")
                    nc.vector.tensor_scalar(
                        hi[:], nmsb[s][:], hi2[:], 230.0, Alu.is_ge, Alu.mult
                    )
                    lo = work.tile([P, W], dt.float32, tag="lo")
                    nc.gpsimd.tensor_scalar(
                        lo[:], nmsb[s][:], lo2[:], 25.0, Alu.is_ge, Alu.mult
                    )
                    ot = work.tile([P, W], dt.float32, tag="ot")
                    nc.vector.tensor_tensor(ot[:], hi[:], lo[:], Alu.add)
                    nc.sync.dma_start(
                        o_d[img, r0 + p0 : r0 + p1, :], ot[p0:p1, :]
                    )

    _split_excess_waits(nc)
    return nc, consts


_CACHE = {}


def _get_program(n_img, reps=1):
    key = (n_img, reps)
    if key not in _CACHE:
        _CACHE[key] = build_program(n_img, reps)
    return _CACHE[key]


def kernel(x: np.ndarray) -> np.ndarray:
    B = x.shape[0]
    n_img = B // N_CORES
    nc, consts = _get_program(n_img)
    xs = np.ascontiguousarray(x.reshape(N_CORES, n_img, H, W).astype(np.float32))
    in_maps = []
    for c in range(N_CORES):
        m = {"x": xs[c]}
        m.update(consts)
        in_maps.append(m)
    res = run_bass_kernel_spmd(nc, in_maps, core_ids=list(range(N_CORES)))
    out = np.stack([res.results[c]["out"] for c in range(N_CORES)], axis=0)
    return out.reshape(B, 1, H, W)


if __name__ == "__main__":
    rng = np.random.default_rng(3)
    x = rng.random((8, 1, H, W), dtype=np.float32)  # 1 img/core quick check
    out = kernel(x)
    sys.path.insert(0, "/root/problem")
    from canny_np import ref_canny_np

    num = den = 0.0
    mism = 0
    for b in range(8):
        e = ref_canny_np(x[b, 0])
        mism += int((out[b, 0] != e).sum())
        num += ((out[b, 0] - e) ** 2).sum()
        den += (e**2).sum()
    print(f"mismatched pixels: {mism} / {8 * H * W}")
    print("rel err:", np.sqrt(num / den))
